# revision 1
# baseline (speedup 1.0000x reference)
"""TRN2 Bass kernel for nn_ST_model_58815282151899 (dense ST-transformer).

Sharding: data-parallel over batch (B=16 -> 2 per core x 8 cores, no collectives).

Per-core layouts:
  node-major (nm):    [128 = n%128, NT=8 = n//128, ... (t, d) free] -- cheb matmuls
                      (contraction over nodes on partitions), attention core
                      (DVE broadcast ops), LN2.
  feature-major (fm): [128 = (j = t%2, d = 64), TP=6 = t//2, n]     -- all linear
                      layers (contraction over features on partitions, t-parity
                      blockdiag pairing to fill 128 partitions), LN1.

Cheb identity (K=4): T0+T1+T2+T3 = 2*A*(T1+T2)  -- telescoped, so only 3
matmul applications of A per support and no accumulator tensor.

All PE operands bf16 (separate LDWEIGHTS path; fp32 matmul is 4x slower and has
a 1-wait-slot codegen hazard). PSUM accumulation fp32. A^T tiles live in DRAM
and are streamed (SBUF budget).
"""
import numpy as np

import concourse.bass as bass
import concourse.bacc as bacc
import concourse.mybir as mybir
from concourse.tile import TileContext
from concourse.masks import make_identity

f32 = mybir.dt.float32
bf16 = mybir.dt.bfloat16
AL = mybir.AluOpType
AF = mybir.ActivationFunctionType
AX = mybir.AxisListType

L, H, EPS = 3, 4, 1e-5
B, T, N, D, F = 16, 12, 1024, 64, 256
HD = D // H           # 16
NCORES = 8
BL = B // NCORES      # 2
NT = N // 128         # 8
TP = T // 2           # 6 t-pairs
TD = T * D            # 768
OS = 12               # out steps

DEBUG_TAPS = ()
CFG = {"at_bufs": 6, "psA_bufs": 3, "offset": 2, "seq": False, "dma_tr_qkv": False}


def _bcast(t_ap, dims, extra_off=0):
    """AP with explicit [step, count] free dims (stride-0 broadcasts allowed)."""
    return bass.AP(t_ap.tensor, t_ap.offset + extra_off,
                   [list(t_ap.ap[0])] + [list(d) for d in dims])


def build_nc():
    nc = bacc.Bacc("TRN2", target_bir_lowering=False, debug=False)

    x_d = nc.dram_tensor("x", [BL, T, N, D], f32, kind="ExternalInput")
    sup_d = nc.dram_tensor("supports", [2, N, N], f32, kind="ExternalInput")
    Wg_d = nc.dram_tensor("Wg", [L, 2 * D, D], f32, kind="ExternalInput")
    bg_d = nc.dram_tensor("bg", [L, D], f32, kind="ExternalInput")
    Wq_d = nc.dram_tensor("Wq", [L, D, D], f32, kind="ExternalInput")
    bq_d = nc.dram_tensor("bq", [L, D], f32, kind="ExternalInput")
    Wk_d = nc.dram_tensor("Wk", [L, D, D], f32, kind="ExternalInput")
    bk_d = nc.dram_tensor("bk", [L, D], f32, kind="ExternalInput")
    Wv_d = nc.dram_tensor("Wv", [L, D, D], f32, kind="ExternalInput")
    bv_d = nc.dram_tensor("bv", [L, D], f32, kind="ExternalInput")
    Wo_d = nc.dram_tensor("Wo", [L, D, D], f32, kind="ExternalInput")
    bo_d = nc.dram_tensor("bo", [L, D], f32, kind="ExternalInput")
    W1_d = nc.dram_tensor("W1", [L, D, F], f32, kind="ExternalInput")
    b1_d = nc.dram_tensor("b1", [L, F], f32, kind="ExternalInput")
    W2_d = nc.dram_tensor("W2", [L, F, D], f32, kind="ExternalInput")
    b2_d = nc.dram_tensor("b2", [L, D], f32, kind="ExternalInput")
    g1_d = nc.dram_tensor("ln1_g", [L, D], f32, kind="ExternalInput")
    be1_d = nc.dram_tensor("ln1_b", [L, D], f32, kind="ExternalInput")
    g2_d = nc.dram_tensor("ln2_g", [L, D], f32, kind="ExternalInput")
    be2_d = nc.dram_tensor("ln2_b", [L, D], f32, kind="ExternalInput")
    Wout_d = nc.dram_tensor("Wout", [TD, OS], f32, kind="ExternalInput")
    bout_d = nc.dram_tensor("bout", [OS], f32, kind="ExternalInput")
    out_d = nc.dram_tensor("out", [BL, OS, N, 1], f32, kind="ExternalOutput")

    taps = {}

    def tap(name, shape, dt=bf16):
        if name is not None and name in DEBUG_TAPS:
            taps[name] = nc.dram_tensor("tap_" + name, shape, dt, kind="ExternalOutput")
            return taps[name]
        return None

    with TileContext(nc) as tc:
        with (
            tc.tile_pool(name="const", bufs=1) as cp,
            tc.tile_pool(name="wp", bufs=1) as wp,
            tc.tile_pool(name="p2", bufs=2) as p2,
            tc.tile_pool(name="p3", bufs=3) as p3,
            tc.tile_pool(name="pat", bufs=CFG["at_bufs"]) as pat,
            tc.tile_pool(name="psA", bufs=CFG["psA_bufs"], space="PSUM") as psA,
            tc.tile_pool(name="psZ", bufs=1, space="PSUM") as psZ,
            tc.tile_pool(name="psT", bufs=2, space="PSUM") as psT,
            tc.tile_pool(name="psS", bufs=2, space="PSUM") as psS,
            tc.tile_pool(name="dramp", bufs=1, space="DRAM") as dramp,
        ):
            # A^T tile store: ATd[s, ks, p, n] = A[s][n, 128*ks + p]
            ATd = dramp.tile([2, NT, 128, N], bf16)
            # ================= constants & weights =================
            ident = cp.tile([128, 128], bf16)
            make_identity(nc, ident[:])

            selS = cp.tile([128, 2], bf16)   # LN1 sum: sel[(j,d), j'] = 1/64 (j==j')
            nc.vector.memset(selS[:], 0.0)
            nc.vector.memset(selS[0:64, 0:1], 1.0 / 64)
            nc.vector.memset(selS[64:128, 1:2], 1.0 / 64)
            # replicate-selector = (selS * 64)^T via PE transpose
            selR = cp.tile([2, 128], bf16)   # sel2[j', (j,d)] = 1 (j==j')
            pselr = psT.tile([128, 128], bf16, tag="trps", name="pselr")[:2]
            nc.tensor.transpose(pselr, selS[:], ident[:])
            nc.scalar.mul(selR[:], pselr, 64.0)

            X = cp.tile([128, NT, BL, T, D], bf16)      # node-major state

            Wg_bd = [cp.tile([128, 2, 128], bf16, name=f"Wgbd{l}") for l in range(L)]
            Wq_bd = [cp.tile([128, 128], bf16, name=f"Wqbd{l}") for l in range(L)]
            Wk_bd = [cp.tile([128, 128], bf16, name=f"Wkbd{l}") for l in range(L)]
            Wv_bd = [cp.tile([128, 128], bf16, name=f"Wvbd{l}") for l in range(L)]
            Wo_bd = [cp.tile([128, 128], bf16, name=f"Wobd{l}") for l in range(L)]
            W1c = [cp.tile([128, 4, 128], bf16, name=f"W1c{l}") for l in range(L)]
            W2c = [cp.tile([128, 4, 128], bf16, name=f"W2c{l}") for l in range(L)]
            Woutc = cp.tile([128, TP, OS], bf16)
            bgv = [cp.tile([128, 1], f32, name=f"bg{l}") for l in range(L)]
            bqv = [cp.tile([128, 1], f32, name=f"bq{l}") for l in range(L)]
            bkv = [cp.tile([128, 1], f32, name=f"bk{l}") for l in range(L)]
            bvv = [cp.tile([128, 1], f32, name=f"bv{l}") for l in range(L)]
            bov = [cp.tile([128, 1], f32, name=f"bo{l}") for l in range(L)]
            b1v = [cp.tile([128, 2], f32, name=f"b1{l}") for l in range(L)]
            b2v = [cp.tile([128, 1], f32, name=f"b2{l}") for l in range(L)]
            g1v = [cp.tile([128, 1], f32, name=f"g1{l}") for l in range(L)]
            be1v = [cp.tile([128, 1], f32, name=f"be1{l}") for l in range(L)]
            boutv = cp.tile([OS, 1], f32)
            epsv = cp.tile([128, 1], f32)
            nc.gpsimd.memset(epsv[:], EPS)
            g2r = [cp.tile([128, D], bf16, name=f"g2r{l}") for l in range(L)]
            be2r = [cp.tile([128, D], bf16, name=f"be2r{l}") for l in range(L)]

            def dup_bias(dst, src_ap):
                nc.gpsimd.dma_start(dst[0:64, :], src_ap[:, None])
                nc.gpsimd.dma_start(dst[64:128, :], src_ap[:, None])

            for l in range(L):
                nc.gpsimd.memset(Wg_bd[l][:], 0.0)
                nc.gpsimd.dma_start(Wg_bd[l][:, 0, 0:64], Wg_d.ap()[l])
                nc.gpsimd.dma_start(Wg_bd[l][:, 1, 64:128], Wg_d.ap()[l])
                for bd, wd in ((Wq_bd, Wq_d), (Wk_bd, Wk_d), (Wv_bd, Wv_d), (Wo_bd, Wo_d)):
                    nc.gpsimd.memset(bd[l][:], 0.0)
                    nc.gpsimd.dma_start(bd[l][0:64, 0:64], wd.ap()[l])
                    nc.gpsimd.dma_start(bd[l][64:128, 64:128], wd.ap()[l])
                nc.gpsimd.memset(W1c[l][:], 0.0)
                nc.gpsimd.dma_start(W1c[l][0:64, 0, :], W1_d.ap()[l, :, 0:128])
                nc.gpsimd.dma_start(W1c[l][0:64, 1, :], W1_d.ap()[l, :, 128:256])
                nc.gpsimd.dma_start(W1c[l][64:128, 2, :], W1_d.ap()[l, :, 0:128])
                nc.gpsimd.dma_start(W1c[l][64:128, 3, :], W1_d.ap()[l, :, 128:256])
                nc.gpsimd.memset(W2c[l][:], 0.0)
                nc.gpsimd.dma_start(W2c[l][:, 0, 0:64], W2_d.ap()[l, 0:128, :])
                nc.gpsimd.dma_start(W2c[l][:, 1, 0:64], W2_d.ap()[l, 128:256, :])
                nc.gpsimd.dma_start(W2c[l][:, 2, 64:128], W2_d.ap()[l, 0:128, :])
                nc.gpsimd.dma_start(W2c[l][:, 3, 64:128], W2_d.ap()[l, 128:256, :])
                dup_bias(bgv[l], bg_d.ap()[l]); dup_bias(bqv[l], bq_d.ap()[l])
                dup_bias(bkv[l], bk_d.ap()[l]); dup_bias(bvv[l], bv_d.ap()[l])
                dup_bias(bov[l], bo_d.ap()[l]); dup_bias(b2v[l], b2_d.ap()[l])
                dup_bias(g1v[l], g1_d.ap()[l]); dup_bias(be1v[l], be1_d.ap()[l])
                nc.gpsimd.dma_start(b1v[l][:, 0:1], b1_d.ap()[l, 0:128][:, None])
                nc.gpsimd.dma_start(b1v[l][:, 1:2], b1_d.ap()[l, 128:256][:, None])
                row = p3.tile([1, D], bf16, tag="ln2row")
                nc.gpsimd.dma_start(row[:], g2_d.ap()[l][None, :])
                nc.gpsimd.partition_broadcast(g2r[l][:], row[:])
                row2 = p3.tile([1, D], bf16, tag="ln2row")
                nc.gpsimd.dma_start(row2[:], be2_d.ap()[l][None, :])
                nc.gpsimd.partition_broadcast(be2r[l][:], row2[:])
            for tpi in range(TP):
                nc.gpsimd.dma_start(
                    Woutc[:, tpi, :],
                    Wout_d.ap().rearrange("(tp p) s -> tp p s", p=128)[tpi])
            nc.gpsimd.dma_start(boutv[:], bout_d.ap()[:, None])

            # ---- x load: f32 stage -> bf16 X
            x_r = x_d.ap().rearrange("b t (nt p) d -> p nt b t d", p=128)
            for ni in range(NT):
                for bi in range(BL):
                    nc.gpsimd.dma_start(X[:, ni, bi], x_r[:, ni, bi])

            # ---- supports: cast+transpose -> ATd (DRAM)
            for s in range(2):
                for nrow in range(NT):
                    a_nm = wp.tile([128, N], bf16, tag="stage0a")
                    nc.gpsimd.dma_start(
                        a_nm[:], sup_d.ap()[s, 128 * nrow:128 * (nrow + 1), :])
                    for ms in range(NT):
                        pt = psT.tile([128, 128], bf16, tag="trps")
                        nc.tensor.transpose(
                            pt[:], a_nm[:, 128 * ms:128 * (ms + 1)], ident[:])
                        att = p3.tile([128, 128], bf16, tag="at_o")
                        nc.scalar.copy(att[:], pt[:])
                        nc.sync.dma_start(
                            ATd[s, ms, :, 128 * nrow:128 * (nrow + 1)], att[:])

            # ================= stages =================
            st = {}

            def stage_A(l, bi):
                """cheb: acc[p, nt, t, s, d] = 2*A_s*(T1+T2)."""
                d = st[(l, bi)] = {}
                acc = wp.tile([128, NT, T, 2, D], bf16, tag="acc16")
                d["acc"] = acc
                rhsX = X[:, :, bi].rearrange("p nt t d -> p nt (t d)")
                for s in range(2):
                    t1 = wp.tile([128, NT, TD], bf16, tag="cheb_t1")
                    t2 = wp.tile([128, NT, TD], bf16, tag="cheb_t2")
                    for term in range(3):
                        rhs = rhsX if term == 0 else t1[:]
                        for ni in range(NT):
                            pm0 = psA.tile([128, 512], f32, tag="mmps", name="pm_cheb0")
                            pm1 = psA.tile([128, 512], f32, tag="mmps", name="pm_cheb1")[:, :256]
                            for ks in range(NT):
                                at = pat.tile([128, 128], bf16, tag="at_s")
                                nc.sync.dma_start(
                                    at[:], ATd[s, ks, :, 128 * ni:128 * (ni + 1)])
                                nc.tensor.matmul(
                                    pm0[:], at[:], rhs[:, ks, 0:512],
                                    start=(ks == 0), stop=(ks == NT - 1))
                                nc.tensor.matmul(
                                    pm1, at[:], rhs[:, ks, 512:768],
                                    start=(ks == 0), stop=(ks == NT - 1))
                            for pm, c0, c1, t0, t1e in ((pm0[:], 0, 512, 0, 8),
                                                        (pm1, 512, 768, 8, 12)):
                                if term == 0:
                                    nc.scalar.copy(t1[:, ni, c0:c1], pm)
                                elif term == 1:
                                    nc.vector.scalar_tensor_tensor(
                                        t2[:, ni, c0:c1], pm, 2.0, rhsX[:, ni, c0:c1],
                                        op0=AL.mult, op1=AL.subtract)
                                else:
                                    nc.scalar.mul(
                                        acc[:, ni, t0:t1e, s, :],
                                        pm.rearrange("p (t d) -> p t d", d=D), 2.0)
                        if term == 1:
                            nc.vector.tensor_tensor(t1[:], t1[:], t2[:], AL.add)
                tp_ = tap("acc0" if (l == 0 and bi == 0) else None, [128, NT * T * 2 * D])
                if tp_ is not None:
                    nc.sync.dma_start(
                        tp_.ap(), acc[:].rearrange("p a b c e -> p (a b c e)"))

            def stage_B(l, bi):
                """acc -> fm -> Wg+relu -> G2; QKV -> nm."""
                d = st[(l, bi)]
                acc = d["acc"]
                G2 = p2.tile([128, TP, N], bf16, tag="G2")
                d["G2"] = G2
                for tpi in range(TP):
                    for ch in range(2):
                        pg = psA.tile([128, 512], f32, tag="mmps")
                        for j in range(2):
                            t_ = 2 * tpi + j
                            fmt = p2.tile([128, 512], bf16, tag="accfm")
                            for w in range(4):
                                ni = 4 * ch + w
                                pt = psT.tile([128, 128], bf16, tag="trps")
                                nc.tensor.transpose(
                                    pt[:],
                                    acc[:, ni, t_].rearrange("p s d -> p (s d)"),
                                    ident[:])
                                nc.scalar.copy(fmt[:, 128 * w:128 * (w + 1)], pt[:])
                            nc.tensor.matmul(pg[:], Wg_bd[l][:, j], fmt[:],
                                             start=(j == 0), stop=(j == 1))
                        nc.scalar.activation(
                            G2[:, tpi, 512 * ch:512 * (ch + 1)], pg[:],
                            AF.Relu, bias=bgv[l][:, 0:1])
                for nm_name, wbd, bv_ in (("q", Wq_bd, bqv), ("k", Wk_bd, bkv),
                                          ("v", Wv_bd, bvv)):
                    nmt = wp.tile([128, NT, T, D], bf16, tag=f"{nm_name}_nm")
                    d[nm_name] = nmt
                    for tpi in range(TP):
                        for ch in range(2):
                            pw = psA.tile([128, 512], f32, tag="mmps")
                            nc.tensor.matmul(
                                pw[:], wbd[l][:],
                                G2[:, tpi, 512 * ch:512 * (ch + 1)],
                                start=True, stop=True)
                            fmw = wp.tile([128, 512], bf16, tag="wf")
                            nc.scalar.activation(fmw[:], pw[:], AF.Identity,
                                                 bias=bv_[l][:, 0:1])
                            for w in range(4):
                                ni = 4 * ch + w
                                dst = (nmt[:, ni, 2 * tpi:2 * tpi + 2, :]
                                       .rearrange("p t d -> p (t d)"))
                                if CFG["dma_tr_qkv"]:
                                    nc.scalar.dma_start_transpose(
                                        dst, fmw[:, 128 * w:128 * (w + 1)])
                                else:
                                    pt = psT.tile([128, 128], bf16, tag="trps")
                                    nc.tensor.transpose(
                                        pt[:], fmw[:, 128 * w:128 * (w + 1)], ident[:])
                                    nc.scalar.copy(dst, pt[:])
                tg = tap(f"G{l}" if bi == 0 else None, [128, TP * N])
                if tg is not None:
                    nc.sync.dma_start(tg.ap(), G2[:].rearrange("p a b -> p (a b)"))
                tq = tap(f"q{l}" if bi == 0 else None, [128, NT * T * D])
                if tq is not None:
                    nc.sync.dma_start(tq.ap(),
                                      d["q"][:].rearrange("p a b c -> p (a b c)"))

            def stage_C(l, bi):
                """attention core (nm, DVE): scores, softmax, AV."""
                d = st[(l, bi)]
                q, k, v = d["q"], d["k"], d["v"]
                o = wp.tile([128, NT, T, D], bf16, tag="o_nm")
                d["o"] = o
                for ni in range(NT):
                    qf = q[:, ni].rearrange("p t d -> p (t d)")
                    kf = k[:, ni].rearrange("p t d -> p (t d)")
                    vf = v[:, ni].rearrange("p t d -> p (t d)")
                    s_t = wp.tile([128, H, T, T], bf16, tag="s_t")
                    for h in range(H):
                        prod = wp.tile([128, T, T, HD], bf16, tag="prodw")
                        q_b = _bcast(qf, [[D, T], [0, T], [1, HD]], HD * h)
                        k_b = _bcast(kf, [[0, T], [D, T], [1, HD]], HD * h)
                        nc.vector.tensor_tensor(prod[:], q_b, k_b, AL.mult)
                        with nc.allow_low_precision(reason="fp32 internal accum"):
                            nc.vector.tensor_reduce(
                                s_t[:, h], prod[:].rearrange("p t t2 hd -> p (t t2) hd"),
                                axis=AX.X, op=AL.add)
                    e_t = wp.tile([128, H, T, T], bf16, tag="e_t")
                    nc.scalar.activation(e_t[:], s_t[:], AF.Exp, scale=1.0 / (HD ** 0.5))
                    den = wp.tile([128, H, T], f32, tag="den")
                    nc.vector.tensor_reduce(den[:], e_t[:], axis=AX.X, op=AL.add)
                    rec = wp.tile([128, H, T], f32, tag="rec")
                    nc.vector.reciprocal_approx_fast(rec[:], den[:])
                    a_t = wp.tile([128, H, T, T], bf16, tag="a_t")
                    rec_b = _bcast(rec[:].rearrange("p h t -> p (h t)"),
                                   [[1, H * T], [0, T]])
                    nc.vector.tensor_tensor(
                        a_t[:].rearrange("p h t t2 -> p (h t) t2"),
                        e_t[:].rearrange("p h t t2 -> p (h t) t2"), rec_b, AL.mult)
                    for h in range(H):
                        prod2 = wp.tile([128, T, HD, T], bf16, tag="prodw")
                        a_b = _bcast(a_t[:, h].rearrange("p t t2 -> p (t t2)"),
                                     [[T, T], [0, HD], [1, T]])
                        v_b = _bcast(vf, [[0, T], [1, HD], [D, T]], HD * h)
                        nc.vector.tensor_tensor(prod2[:], a_b, v_b, AL.mult)
                        with nc.allow_low_precision(reason="DVE reduce is fp32 internal"):
                            nc.vector.tensor_reduce(
                                o[:, ni, :, HD * h:HD * (h + 1)],
                                prod2[:].rearrange("p t hd t2 -> p (t hd) t2"),
                                axis=AX.X, op=AL.add)
                ts_ = tap(f"o{l}" if bi == 0 else None, [128, NT * T * D])
                if ts_ is not None:
                    nc.sync.dma_start(ts_.ap(), o[:].rearrange("p a b c -> p (a b c)"))

            def stage_D(l, bi):
                """Wo+res; LN1 (fm, PE-stats); FFN+res; -> nm; LN2 -> X."""
                d = st[(l, bi)]
                G2, o = d["G2"], d["o"]
                x1n = wp.tile([128, TP, N], bf16, tag="x1n")
                for tpi in range(TP):
                    for ch in range(2):
                        ofm = p2.tile([128, 512], bf16, tag="accfm")
                        for w in range(4):
                            ni = 4 * ch + w
                            pt = psT.tile([128, 128], bf16, tag="trps")
                            nc.tensor.transpose(
                                pt[:],
                                o[:, ni, 2 * tpi:2 * tpi + 2, :]
                                .rearrange("p t d -> p (t d)"),
                                ident[:])
                            nc.scalar.copy(ofm[:, 128 * w:128 * (w + 1)], pt[:])
                        po = psA.tile([128, 512], f32, tag="mmps")
                        nc.tensor.matmul(po[:], Wo_bd[l][:], ofm[:],
                                         start=True, stop=True)
                        g2s = G2[:, tpi, 512 * ch:512 * (ch + 1)]
                        # x1 = G2 + (wo_out + bo)   (in-place)
                        nc.vector.scalar_tensor_tensor(
                            g2s, po[:], bov[l][:, 0:1], g2s, op0=AL.add, op1=AL.add)
                        # LN1 stats via ones-matmuls
                        sq = wp.tile([128, 512], bf16, tag="sq")
                        nc.scalar.square(sq[:], g2s)
                        pm_ = psS.tile([128, 512], f32, tag="stps", name="pm_st")[:2]
                        px2 = psS.tile([128, 512], f32, tag="stps", name="px2_st")[:2]
                        nc.tensor.matmul(pm_, selS[:], g2s, start=True, stop=True)
                        nc.tensor.matmul(px2, selS[:], sq[:], start=True, stop=True)
                        m_sb = wp.tile([2, 512], bf16, tag="m_sb")
                        nc.vector.tensor_copy(m_sb[:], pm_)
                        m2 = wp.tile([2, 512], f32, tag="m2")
                        nc.vector.tensor_tensor(m2[:], m_sb[:], m_sb[:], AL.mult)
                        var = wp.tile([2, 512], f32, tag="var")
                        nc.vector.tensor_tensor(var[:], px2, m2[:], AL.subtract)
                        sd = wp.tile([2, 512], f32, tag="sd")
                        nc.scalar.activation(sd[:], var[:], AF.Sqrt, bias=epsv[:2, 0:1])
                        rstdf = wp.tile([2, 512], f32, tag="rstdf")
                        nc.vector.reciprocal_approx_fast(rstdf[:], sd[:])
                        rstd = wp.tile([2, 512], bf16, tag="rstd")
                        nc.vector.tensor_copy(rstd[:], rstdf[:])
                        pmr = psS.tile([128, 512], f32, tag="stps")
                        nc.tensor.matmul(pmr[:], selR[:], m_sb[:], start=True, stop=True)
                        prr = psS.tile([128, 512], f32, tag="stps")
                        nc.tensor.matmul(prr[:], selR[:], rstd[:], start=True, stop=True)
                        cen = wp.tile([128, 512], bf16, tag="cen")
                        nc.vector.tensor_tensor(cen[:], g2s, pmr[:], AL.subtract)
                        xh = wp.tile([128, 512], bf16, tag="xh")
                        nc.vector.tensor_tensor(xh[:], cen[:], prr[:], AL.mult)
                        x1ns = x1n[:, tpi, 512 * ch:512 * (ch + 1)]
                        nc.scalar.activation(x1ns, xh[:], AF.Identity,
                                             bias=be1v[l][:, 0:1], scale=g1v[l][:, 0:1])
                        # FFN
                        pz = psZ.tile([128, 512], f32, tag="zps")
                        for c in range(4):
                            pmid = psA.tile([128, 512], f32, tag="mmps")
                            nc.tensor.matmul(pmid[:], W1c[l][:, c], x1ns,
                                             start=True, stop=True)
                            mid = wp.tile([128, 512], bf16, tag="mid")
                            nc.scalar.activation(mid[:], pmid[:], AF.Relu,
                                                 bias=b1v[l][:, c % 2:c % 2 + 1])
                            nc.tensor.matmul(pz[:], W2c[l][:, c], mid[:],
                                             start=(c == 0), stop=(c == 3))
                        # z = x1n + (w2_out + b2)  (stored into G2 slot)
                        nc.vector.scalar_tensor_tensor(
                            g2s, pz[:], b2v[l][:, 0:1], x1ns, op0=AL.add, op1=AL.add)
                # z -> nm ; LN2 per ni -> X
                for ni in range(NT):
                    z_nm = wp.tile([128, T, D], bf16, tag="z_nm")
                    for tpi in range(TP):
                        pt = psT.tile([128, 128], bf16, tag="trps")
                        nc.tensor.transpose(
                            pt[:], G2[:, tpi, 128 * ni:128 * (ni + 1)], ident[:])
                        nc.scalar.copy(
                            z_nm[:, 2 * tpi:2 * tpi + 2, :]
                            .rearrange("p t d -> p (t d)"),
                            pt[:])
                    m_ = p3.tile([128, T], f32, tag="ln2m")
                    nc.vector.tensor_reduce(m_[:], z_nm[:], axis=AX.X, op=AL.add)
                    nc.scalar.mul(m_[:], m_[:], 1.0 / D)
                    cen2 = wp.tile([128, T, D], bf16, tag="ln2cen")
                    m_b = _bcast(m_[:], [[1, T], [0, D]])
                    nc.vector.tensor_tensor(cen2[:], z_nm[:], m_b, AL.subtract)
                    sq2 = wp.tile([128, T, D], bf16, tag="ln2sq")
                    nc.scalar.square(sq2[:], cen2[:])
                    var2 = p3.tile([128, T], f32, tag="ln2var")
                    nc.vector.tensor_reduce(var2[:], sq2[:], axis=AX.X, op=AL.add)
                    sd2 = p3.tile([128, T], f32, tag="ln2sd")
                    nc.scalar.activation(sd2[:], var2[:], AF.Sqrt,
                                         scale=1.0 / D, bias=epsv[:, 0:1])
                    rstd2 = p3.tile([128, T], f32, tag="ln2rstd")
                    nc.vector.reciprocal_approx_fast(rstd2[:], sd2[:])
                    xh2 = wp.tile([128, T, D], bf16, tag="ln2xh")
                    r_b = _bcast(rstd2[:], [[1, T], [0, D]])
                    nc.vector.tensor_tensor(xh2[:], cen2[:], r_b, AL.mult)
                    g_b = _bcast(g2r[l][:], [[0, T], [1, D]])
                    nc.vector.tensor_tensor(xh2[:], xh2[:], g_b, AL.mult)
                    b_b = _bcast(be2r[l][:], [[0, T], [1, D]])
                    nc.vector.tensor_tensor(X[:, ni, bi], xh2[:], b_b, AL.add)
                tx = tap(f"X{l}b{bi}", [128, NT * T * D])
                if tx is not None:
                    nc.sync.dma_start(
                        tx.ap().rearrange("p (nt td) -> p nt td", nt=NT),
                        X[:, :, bi].rearrange("p nt t d -> p nt (t d)"))

            def stage_F(bi):
                xf = wp.tile([128, TP, N], bf16, tag="x1n")
                for tpi in range(TP):
                    for ni in range(NT):
                        pt = psT.tile([128, 128], bf16, tag="trps")
                        nc.tensor.transpose(
                            pt[:],
                            X[:, ni, bi, 2 * tpi:2 * tpi + 2, :]
                            .rearrange("p t d -> p (t d)"),
                            ident[:])
                        nc.scalar.copy(xf[:, tpi, 128 * ni:128 * (ni + 1)], pt[:])
                outsb = wp.tile([OS, N], bf16, tag="outsb")
                for ch in range(2):
                    pf = psA.tile([128, 512], f32, tag="mmps", name="pf_out")[:OS]
                    for tpi in range(TP):
                        nc.tensor.matmul(
                            pf, Woutc[:, tpi], xf[:, tpi, 512 * ch:512 * (ch + 1)],
                            start=(tpi == 0), stop=(tpi == TP - 1))
                    nc.scalar.activation(outsb[:, 512 * ch:512 * (ch + 1)], pf,
                                         AF.Identity, bias=boutv[:, 0:1])
                nc.gpsimd.dma_start(out_d.ap()[bi].rearrange("s n o -> s (n o)"),
                                    outsb[:])

            # -------- emission: 2-stream interleave --------
            prog = {b: [] for b in range(BL)}
            for l in range(L):
                for b in range(BL):
                    prog[b] += [(stage_A, l, b), (stage_B, l, b),
                                (stage_C, l, b), (stage_D, l, b)]
            for b in range(BL):
                prog[b].append((stage_F, b))
            if CFG["seq"]:
                order = prog[0] + prog[1]
            else:
                order = []
                i0 = i1 = 0
                OFFSET = CFG["offset"]
                while i0 < len(prog[0]) or i1 < len(prog[1]):
                    if i0 < len(prog[0]) and (i0 - OFFSET < i1 or i1 >= len(prog[1])):
                        order.append(prog[0][i0]); i0 += 1
                    else:
                        order.append(prog[1][i1]); i1 += 1
            for fn, *args in order:
                fn(*args)

    nc.compile()
    return nc, taps


_CACHE = {}


def _get_nc():
    if "nc" not in _CACHE:
        _CACHE["nc"] = build_nc()
    return _CACHE["nc"]


def kernel(**inputs):
    from concourse.bass_utils import run_bass_kernel_spmd
    nc, taps = _get_nc()
    x = np.asarray(inputs["x"], dtype=np.float32)
    names = ["supports", "Wg", "bg", "Wq", "bq", "Wk", "bk", "Wv", "bv", "Wo", "bo",
             "W1", "b1", "W2", "b2", "ln1_g", "ln1_b", "ln2_g", "ln2_b",
             "Wout", "bout"]
    shared = {n: np.ascontiguousarray(np.asarray(inputs[n], dtype=np.float32))
              for n in names}
    in_maps = []
    for c in range(NCORES):
        m = dict(shared)
        m["x"] = np.ascontiguousarray(x[c * BL:(c + 1) * BL])
        in_maps.append(m)
    res = run_bass_kernel_spmd(nc, in_maps, core_ids=list(range(NCORES)))
    _CACHE["last_res"] = res
    out = np.concatenate([r["out"] for r in res.results], axis=0)
    return out.astype(np.float32)



# revision 8
# speedup vs baseline: 1.3292x; 1.3292x over previous
"""TRN2 Bass kernel for nn_ST_model_58815282151899 (dense ST-transformer).

Sharding: data-parallel over batch (B=16 -> 2 per core x 8 cores, no collectives).

Key structure (vs naive):
  * Chebyshev collapse: sum_{k<4} T_k(A) = 4A^3 + 2A^2 - 2A =: M (per support).
    M~ = [M1 M2] is precomputed ONCE on device and kept SBUF-resident (bf16).
    Per layer the GNN is then   G = relu(M1 (x Wg1) + M2 (x Wg2) + bg)
    = one 2048-deep PSUM-accumulated matmul (feature transform applied first).
  * Layouts: feature-major fm = [(j=t%2, d) part, tp=t//2, n] for all linears
    and both layernorms (PE ones-matmul stats); node-major nm = [n%128 part,
    (t, d) free] only for the attention core (DVE broadcast ops).
  * The cheb matmul uses transposed-u tiles as PE *stationary* and M~^T as
    moving operand, so its output lands directly in fm (no nm->fm transposes).
  * q/k/v are produced directly in nm by using G2 tiles as stationary and the
    block-diagonal Wq/Wk/Wv as moving operand.
  * Softmax normalization folded into o (scale by 1/den once per node tile).
  * x is pre-transposed to fm and cast bf16 on host; A and A^T passed bf16.
"""
import numpy as np

import concourse.bass as bass
import concourse.bacc as bacc
import concourse.mybir as mybir
from concourse.tile import TileContext
from concourse.masks import make_identity

f32 = mybir.dt.float32
bf16 = mybir.dt.bfloat16
AL = mybir.AluOpType
AF = mybir.ActivationFunctionType
AX = mybir.AxisListType

L, H, EPS = 3, 4, 1e-5
B, T, N, D, F = 16, 12, 1024, 64, 256
HD = D // H           # 16
NCORES = 8
BL = B // NCORES      # 2
NT = N // 128         # 8
TP = T // 2           # 6 t-pairs
TD = T * D            # 768
OS = 12               # out steps

DEBUG_TAPS = ()
CFG = {"offset": 2, "seq": False}


def _bcast(t_ap, dims, extra_off=0):
    """AP with explicit [step, count] free dims (stride-0 broadcasts allowed)."""
    return bass.AP(t_ap.tensor, t_ap.offset + extra_off,
                   [list(t_ap.ap[0])] + [list(d) for d in dims])


def build_nc(qkv_bias=False):
    nc = bacc.Bacc("TRN2", target_bir_lowering=False, debug=False)

    # x pre-transposed to fm on host: [BL, 128=(j,d), TP, N] bf16
    x_d = nc.dram_tensor("x", [BL, 128, TP, N], bf16, kind="ExternalInput")
    a_d = nc.dram_tensor("a_bf", [2, N, N], bf16, kind="ExternalInput")
    at_d = nc.dram_tensor("at_bf", [2, N, N], bf16, kind="ExternalInput")
    Wg_d = nc.dram_tensor("Wg", [L, 2 * D, D], f32, kind="ExternalInput")
    bg_d = nc.dram_tensor("bg", [L, D], f32, kind="ExternalInput")
    Wq_d = nc.dram_tensor("Wq", [L, D, D], f32, kind="ExternalInput")
    Wk_d = nc.dram_tensor("Wk", [L, D, D], f32, kind="ExternalInput")
    Wv_d = nc.dram_tensor("Wv", [L, D, D], f32, kind="ExternalInput")
    Wo_d = nc.dram_tensor("Wo", [L, D, D], f32, kind="ExternalInput")
    bo_d = nc.dram_tensor("bo", [L, D], f32, kind="ExternalInput")
    W1_d = nc.dram_tensor("W1", [L, D, F], f32, kind="ExternalInput")
    b1_d = nc.dram_tensor("b1", [L, F], f32, kind="ExternalInput")
    W2_d = nc.dram_tensor("W2", [L, F, D], f32, kind="ExternalInput")
    b2_d = nc.dram_tensor("b2", [L, D], f32, kind="ExternalInput")
    g1_d = nc.dram_tensor("ln1_g", [L, D], f32, kind="ExternalInput")
    be1_d = nc.dram_tensor("ln1_b", [L, D], f32, kind="ExternalInput")
    g2_d = nc.dram_tensor("ln2_g", [L, D], f32, kind="ExternalInput")
    be2_d = nc.dram_tensor("ln2_b", [L, D], f32, kind="ExternalInput")
    Wout_d = nc.dram_tensor("Wout", [TD, OS], f32, kind="ExternalInput")
    bout_d = nc.dram_tensor("bout", [OS], f32, kind="ExternalInput")
    bqkv_d = nc.dram_tensor("bqkv", [L, 3, D], f32, kind="ExternalInput")
    out_d = nc.dram_tensor("out", [BL, OS, N, 1], f32, kind="ExternalOutput")

    taps = {}

    def tap(name, shape, dt=bf16):
        if name is not None and name in DEBUG_TAPS:
            taps[name] = nc.dram_tensor("tap_" + name, shape, dt, kind="ExternalOutput")
            return taps[name]
        return None

    with TileContext(nc) as tc:
        with (
            tc.tile_pool(name="const", bufs=1) as cp,
            tc.tile_pool(name="wp", bufs=1) as wp,
            tc.tile_pool(name="p2", bufs=2) as p2,
            tc.tile_pool(name="p3", bufs=2) as p3,
            tc.tile_pool(name="pLN", bufs=1) as pLN,
            tc.tile_pool(name="pat", bufs=4) as pat,
            tc.tile_pool(name="psA", bufs=3, space="PSUM") as psA,
            tc.tile_pool(name="psQ", bufs=1, space="PSUM") as psQ,
            tc.tile_pool(name="psZ", bufs=1, space="PSUM") as psZ,
            tc.tile_pool(name="psT", bufs=1, space="PSUM") as psT,
            tc.tile_pool(name="psS", bufs=2, space="PSUM") as psS,
        ):
            # ================= persistent SBUF =================
            Bt = cp.tile([128, 2, NT, N], bf16)          # M~^T tiles (moving)
            X = cp.tile([128, BL, TP, N], bf16)          # fm state
            UNM = cp.tile([128, 2, NT, TP, 128], bf16)   # u^T tiles (stationary)
            G2 = [cp.tile([128, TP, N], bf16, name=f"G2_{b}") for b in range(BL)]
            ONM = [cp.tile([128, NT, T, D], bf16, name=f"o_{b}") for b in range(BL)]
            X1N = cp.tile([128, TP, N], bf16)            # LN1 out (PE-phase only)

            ident = cp.tile([128, 128], bf16)
            make_identity(nc, ident[:])

            selS = cp.tile([128, 2], bf16)   # LN sum: sel[(j,d), j'] = 1/64 (j==j')
            nc.vector.memset(selS[:], 0.0)
            nc.vector.memset(selS[0:64, 0:1], 1.0 / 64)
            nc.vector.memset(selS[64:128, 1:2], 1.0 / 64)
            selR = cp.tile([2, 128], bf16)   # replicate: sel2[j', (j,d)] = 1 (j==j')
            pselr = psT.tile([128, 512], bf16, tag="trps", name="pselr")[:, :128]
            nc.tensor.transpose(pselr[:2], selS[:], ident[:])
            nc.scalar.mul(selR[:], pselr[:2], 64.0)

            # ---- weights ----
            Wg_bd = [[cp.tile([128, 128], bf16, name=f"Wgbd{l}_{s}") for s in range(2)]
                     for l in range(L)]
            Wq_bd = [cp.tile([128, 128], bf16, name=f"Wqbd{l}") for l in range(L)]
            Wk_bd = [cp.tile([128, 128], bf16, name=f"Wkbd{l}") for l in range(L)]
            Wv_bd = [cp.tile([128, 128], bf16, name=f"Wvbd{l}") for l in range(L)]
            Wo_bd = [cp.tile([128, 128], bf16, name=f"Wobd{l}") for l in range(L)]
            W1c = [cp.tile([128, 4, 128], bf16, name=f"W1c{l}") for l in range(L)]
            W2c = [cp.tile([128, 4, 128], bf16, name=f"W2c{l}") for l in range(L)]
            Woutc = cp.tile([128, TP, OS], bf16)
            bgv = [cp.tile([128, 1], f32, name=f"bg{l}") for l in range(L)]
            bov = [cp.tile([128, 1], f32, name=f"bo{l}") for l in range(L)]
            b1v = [cp.tile([128, 2], f32, name=f"b1{l}") for l in range(L)]
            b2v = [cp.tile([128, 1], f32, name=f"b2{l}") for l in range(L)]
            g1v = [cp.tile([128, 1], f32, name=f"g1{l}") for l in range(L)]
            be1v = [cp.tile([128, 1], f32, name=f"be1{l}") for l in range(L)]
            g2v = [cp.tile([128, 1], f32, name=f"g2{l}") for l in range(L)]
            be2v = [cp.tile([128, 1], f32, name=f"be2{l}") for l in range(L)]
            boutv = cp.tile([OS, 1], f32)
            epsv = cp.tile([128, 1], f32)
            nc.gpsimd.memset(epsv[:], EPS)
            if qkv_bias:
                bqkvr = [cp.tile([128, 3, 2, D], bf16, name=f"bqkv{l}")
                         for l in range(L)]

            def dup_bias(dst, src_ap):
                nc.gpsimd.dma_start(dst[0:64, :], src_ap[:, None])
                nc.gpsimd.dma_start(dst[64:128, :], src_ap[:, None])

            for l in range(L):
                for s in range(2):
                    nc.gpsimd.memset(Wg_bd[l][s][:], 0.0)
                    nc.gpsimd.dma_start(Wg_bd[l][s][0:64, 0:64],
                                        Wg_d.ap()[l, 64 * s:64 * (s + 1), :])
                    nc.gpsimd.dma_start(Wg_bd[l][s][64:128, 64:128],
                                        Wg_d.ap()[l, 64 * s:64 * (s + 1), :])
                for bd, wd in ((Wq_bd, Wq_d), (Wk_bd, Wk_d), (Wv_bd, Wv_d),
                               (Wo_bd, Wo_d)):
                    nc.gpsimd.memset(bd[l][:], 0.0)
                    nc.gpsimd.dma_start(bd[l][0:64, 0:64], wd.ap()[l])
                    nc.gpsimd.dma_start(bd[l][64:128, 64:128], wd.ap()[l])
                nc.gpsimd.memset(W1c[l][:], 0.0)
                nc.gpsimd.dma_start(W1c[l][0:64, 0, :], W1_d.ap()[l, :, 0:128])
                nc.gpsimd.dma_start(W1c[l][0:64, 1, :], W1_d.ap()[l, :, 128:256])
                nc.gpsimd.dma_start(W1c[l][64:128, 2, :], W1_d.ap()[l, :, 0:128])
                nc.gpsimd.dma_start(W1c[l][64:128, 3, :], W1_d.ap()[l, :, 128:256])
                nc.gpsimd.memset(W2c[l][:], 0.0)
                nc.gpsimd.dma_start(W2c[l][:, 0, 0:64], W2_d.ap()[l, 0:128, :])
                nc.gpsimd.dma_start(W2c[l][:, 1, 0:64], W2_d.ap()[l, 128:256, :])
                nc.gpsimd.dma_start(W2c[l][:, 2, 64:128], W2_d.ap()[l, 0:128, :])
                nc.gpsimd.dma_start(W2c[l][:, 3, 64:128], W2_d.ap()[l, 128:256, :])
                dup_bias(bgv[l], bg_d.ap()[l]); dup_bias(bov[l], bo_d.ap()[l])
                dup_bias(b2v[l], b2_d.ap()[l]); dup_bias(g1v[l], g1_d.ap()[l])
                dup_bias(be1v[l], be1_d.ap()[l]); dup_bias(g2v[l], g2_d.ap()[l])
                dup_bias(be2v[l], be2_d.ap()[l])
                nc.gpsimd.dma_start(b1v[l][:, 0:1], b1_d.ap()[l, 0:128][:, None])
                nc.gpsimd.dma_start(b1v[l][:, 1:2], b1_d.ap()[l, 128:256][:, None])
                if qkv_bias:
                    row = p3.tile([1, 3 * D], bf16, tag="bqrow")
                    nc.gpsimd.dma_start(
                        row[:], bqkv_d.ap()[l].rearrange("w d -> (w d)")[None, :])
                    for j in range(2):
                        nc.gpsimd.partition_broadcast(
                            bqkvr[l][:, :, j, :].rearrange("p w d -> p (w d)"),
                            row[:])
            for tpi in range(TP):
                nc.gpsimd.dma_start(
                    Woutc[:, tpi, :],
                    Wout_d.ap().rearrange("(tp p) s -> tp p s", p=128)[tpi])
            nc.gpsimd.dma_start(boutv[:], bout_d.ap()[:, None])

            # ---- x load (already fm bf16 on host)
            for bi in range(BL):
                nc.sync.dma_start(
                    X[:, bi].rearrange("p tp n -> p (tp n)"),
                    x_d.ap()[bi].rearrange("p tp n -> p (tp n)"))

            # ================= B = M~^T precompute =================
            # C := A^T (per support).  C2 = C @ C, C3 = C @ C2, computed with
            # natural-A tiles as stationary:  (C@Y)[i,n] = sum_k A[k,i] Y[k,n].
            # B'_s = 2*C3 + C2 - C   (x2 folded into the G2 relu scale).
            atbuf = (UNM[:].rearrange("p s kb t n -> p (s kb t n)")[:, 0:NT * N]
                     .rearrange("p (kb n) -> p kb n", n=N))
            def c2ap(s, it, sl):
                if s == 0:
                    return Bt[:, 1, it, sl]
                return (G2[0][:, it, sl] if it < TP
                        else G2[1][:, it - TP, sl])
            for s in range(2):
                nc.sync.dma_start(
                    atbuf, at_d.ap()[s].rearrange("(kb p) n -> p kb n", p=128))
                for pass_i in range(2):  # 0: C2 = C@C, 1: B = 2*C@C2 + C2 - C
                    for it in range(NT):
                        ps0 = psA.tile([128, 512], f32, tag="mmps", name="pb0")
                        ps1 = psA.tile([128, 512], f32, tag="mmps", name="pb1")
                        for kt in range(NT):
                            an = pat.tile([128, 128], bf16, tag="a_nat")
                            nc.sync.dma_start(
                                an[:], a_d.ap()[s, 128 * kt:128 * (kt + 1),
                                                128 * it:128 * (it + 1)])
                            for half, ps in ((0, ps0), (1, ps1)):
                                sl = slice(512 * half, 512 * (half + 1))
                                rhs = (atbuf[:, kt, sl] if pass_i == 0
                                       else c2ap(s, kt, sl))
                                nc.tensor.matmul(ps[:], an[:], rhs,
                                                 start=(kt == 0), stop=(kt == NT - 1))
                        for half, ps in ((0, ps0), (1, ps1)):
                            sl = slice(512 * half, 512 * (half + 1))
                            if pass_i == 0:
                                nc.scalar.copy(c2ap(s, it, sl), ps[:])
                            else:
                                tmp = p3.tile([128, 512], bf16, tag="bcomb")
                                nc.vector.tensor_tensor(
                                    tmp[:], c2ap(s, it, sl), atbuf[:, it, sl],
                                    AL.subtract)
                                nc.vector.scalar_tensor_tensor(
                                    Bt[:, s, it, sl], ps[:], 2.0, tmp[:],
                                    op0=AL.mult, op1=AL.add)
            tb = tap("Bt", [128, 2 * NT * N])
            if tb is not None:
                nc.sync.dma_start(tb.ap(), Bt[:].rearrange("p a b c -> p (a b c)"))

            # ================= stages =================
            st = {}

            def stage_A(l, bi):
                """u_s = X@Wg_s (fm) -> transpose tiles -> UNM; then cheb:
                G2 = relu(2 * B'^T-contraction + bg) directly in fm."""
                for s in range(2):
                    for tpi in range(TP):
                        for ch in range(2):
                            pu = psA.tile([128, 512], f32, tag="mmps", name="pu")
                            nc.tensor.matmul(
                                pu[:], Wg_bd[l][s][:],
                                X[:, bi, tpi, 512 * ch:512 * (ch + 1)],
                                start=True, stop=True)
                            ufm = p3.tile([128, 512], bf16, tag="ufm")
                            nc.scalar.copy(ufm[:], pu[:])
                            pt = psT.tile([128, 512], bf16, tag="trps")
                            for w in range(4):
                                nc.tensor.transpose(
                                    pt[:, 128 * w:128 * (w + 1)],
                                    ufm[:, 128 * w:128 * (w + 1)], ident[:])
                            nc.scalar.copy(
                                UNM[:, s, 4 * ch:4 * ch + 4, tpi, :],
                                pt[:].rearrange("p (kb n) -> p kb n", n=128))
                for tpi in range(TP):
                    ps0 = psA.tile([128, 512], f32, tag="mmps", name="pc0")
                    ps1 = psA.tile([128, 512], f32, tag="mmps", name="pc1")
                    for s in range(2):
                        for kb in range(NT):
                            first = (s == 0 and kb == 0)
                            last = (s == 1 and kb == NT - 1)
                            lhs = UNM[:, s, kb, tpi, :]
                            nc.tensor.matmul(ps0[:], lhs, Bt[:, s, kb, 0:512],
                                             start=first, stop=last)
                            nc.tensor.matmul(ps1[:], lhs, Bt[:, s, kb, 512:1024],
                                             start=first, stop=last)
                    nc.scalar.activation(G2[bi][:, tpi, 0:512], ps0[:],
                                         AF.Relu, bias=bgv[l][:, 0:1], scale=2.0)
                    nc.scalar.activation(G2[bi][:, tpi, 512:1024], ps1[:],
                                         AF.Relu, bias=bgv[l][:, 0:1], scale=2.0)
                tg = tap(f"G{l}" if bi == 0 else None, [128, TP * N])
                if tg is not None:
                    nc.sync.dma_start(tg.ap(),
                                      G2[bi][:].rearrange("p a b -> p (a b)"))

            def stage_QC(l, bi):
                """Per node-tile: qkv direct into nm, then attention core."""
                o = ONM[bi]
                for ni in range(NT):
                    # ---- qkv: out[n, (j,d')] per tpi; psq free = (w, j, d')
                    q = p2.tile([128, T, D], bf16, tag=f"q{bi}")
                    k = p2.tile([128, T, D], bf16, tag=f"k{bi}")
                    v = p2.tile([128, T, D], bf16, tag=f"v{bi}")
                    for tpi in range(TP):
                        psq = psQ.tile([128, 3, 2, 64], f32, tag="psq")
                        g2t = G2[bi][:, tpi, 128 * ni:128 * (ni + 1)]
                        for w, wbd in enumerate((Wq_bd, Wk_bd, Wv_bd)):
                            nc.tensor.matmul(
                                psq[:, w].rearrange("p j d -> p (j d)"),
                                g2t, wbd[l][:], start=True, stop=True)
                        for w, dst in enumerate((q, k, v)):
                            d_ap = dst[:, 2 * tpi:2 * tpi + 2, :]
                            if qkv_bias:
                                nc.vector.tensor_tensor(
                                    d_ap, psq[:, w], bqkvr[l][:, w], AL.add)
                            else:
                                nc.scalar.copy(d_ap, psq[:, w])
                    # ---- attention core (DVE)
                    qf = q[:].rearrange("p t d -> p (t d)")
                    kf = k[:].rearrange("p t d -> p (t d)")
                    vf = v[:].rearrange("p t d -> p (t d)")
                    s_t = wp.tile([128, H, T, T], bf16, tag=f"s_t{bi}")
                    for h in range(H):
                        prod = wp.tile([128, T, T, HD], bf16, tag=f"prodw{bi}")
                        q_b = _bcast(qf, [[D, T], [0, T], [1, HD]], HD * h)
                        k_b = _bcast(kf, [[0, T], [D, T], [1, HD]], HD * h)
                        nc.vector.tensor_tensor(prod[:], q_b, k_b, AL.mult)
                        with nc.allow_low_precision(reason="fp32 internal accum"):
                            nc.vector.tensor_reduce(
                                s_t[:, h],
                                prod[:].rearrange("p t t2 hd -> p (t t2) hd"),
                                axis=AX.X, op=AL.add)
                    e_t = wp.tile([128, H, T, T], bf16, tag=f"e_t{bi}")
                    nc.scalar.activation(e_t[:], s_t[:], AF.Exp,
                                         scale=1.0 / (HD ** 0.5))
                    den = wp.tile([128, H, T], f32, tag=f"den{bi}")
                    nc.vector.tensor_reduce(den[:], e_t[:], axis=AX.X, op=AL.add)
                    rec = wp.tile([128, H, T], f32, tag=f"rec{bi}")
                    nc.vector.reciprocal_approx_fast(rec[:], den[:])
                    recb = wp.tile([128, H, T], bf16, tag=f"recb{bi}")
                    nc.vector.tensor_copy(recb[:], rec[:])
                    for h in range(H):
                        prod2 = wp.tile([128, T, HD, T], bf16, tag=f"prodw{bi}")
                        e_b = _bcast(e_t[:, h].rearrange("p t t2 -> p (t t2)"),
                                     [[T, T], [0, HD], [1, T]])
                        v_b = _bcast(vf, [[0, T], [1, HD], [D, T]], HD * h)
                        nc.vector.tensor_tensor(prod2[:], e_b, v_b, AL.mult)
                        with nc.allow_low_precision(reason="fp32 internal accum"):
                            nc.vector.tensor_reduce(
                                o[:, ni, :, HD * h:HD * (h + 1)],
                                prod2[:].rearrange("p t hd t2 -> p (t hd) t2"),
                                axis=AX.X, op=AL.add)
                    # o *= 1/den  (softmax normalization folded here)
                    r_b = _bcast(recb[:].rearrange("p h t -> p (h t)"),
                                 [[1, T], [T, H], [0, HD]])
                    of = o[:, ni].rearrange("p t d -> p (t d)")
                    o3 = bass.AP(of.tensor, of.offset,
                                 [list(of.ap[0]), [D, T], [HD, H], [1, HD]])
                    nc.vector.tensor_tensor(o3, o3, r_b, AL.mult)
                ts_ = tap(f"o{l}" if bi == 0 else None, [128, NT * T * D])
                if ts_ is not None:
                    nc.sync.dma_start(ts_.ap(),
                                      o[:].rearrange("p a b c -> p (a b c)"))

            def _ln_fm(z_chunk, x1_dst, gv, bev, tag):
                """Post-LN in fm on a [128, 512] chunk: PE ones-matmul stats."""
                sq = pLN.tile([128, 512], bf16, tag="sq")
                nc.scalar.square(sq[:], z_chunk)
                pm_ = psS.tile([128, 512], f32, tag="stps", name="pm_st")[:2]
                px2 = psS.tile([128, 512], f32, tag="stps", name="px2_st")[:2]
                nc.tensor.matmul(pm_, selS[:], z_chunk, start=True, stop=True)
                nc.tensor.matmul(px2, selS[:], sq[:], start=True, stop=True)
                m_sb = pLN.tile([2, 512], bf16, tag="m_sb")
                nc.vector.tensor_copy(m_sb[:], pm_)
                stat = pLN.tile([2, 512], f32, tag="stat")
                nc.vector.tensor_tensor(stat[:], m_sb[:], m_sb[:], AL.mult)
                nc.vector.tensor_tensor(stat[:], px2, stat[:], AL.subtract)
                sd = pLN.tile([2, 512], f32, tag="sd")
                nc.scalar.activation(sd[:], stat[:], AF.Sqrt, bias=epsv[:2, 0:1])
                nc.vector.reciprocal_approx_fast(stat[:], sd[:])
                rstd = pLN.tile([2, 512], bf16, tag="rstd")
                nc.vector.tensor_copy(rstd[:], stat[:])
                pmr = psS.tile([128, 512], f32, tag="stps")
                nc.tensor.matmul(pmr[:], selR[:], m_sb[:], start=True, stop=True)
                prr = psS.tile([128, 512], f32, tag="stps")
                nc.tensor.matmul(prr[:], selR[:], rstd[:], start=True, stop=True)
                cen = pLN.tile([128, 512], bf16, tag="cen")
                nc.vector.tensor_tensor(cen[:], z_chunk, pmr[:], AL.subtract)
                xh = pLN.tile([128, 512], bf16, tag="xh")
                nc.vector.tensor_tensor(xh[:], cen[:], prr[:], AL.mult)
                nc.scalar.activation(x1_dst, xh[:], AF.Identity,
                                     bias=bev[:, 0:1], scale=gv[:, 0:1])

            def stage_D(l, bi):
                """o->fm; Wo+res; LN1; FFN+res; LN2 -> X (all fm)."""
                o, g2 = ONM[bi], G2[bi]
                for tpi in range(TP):
                    for ch in range(2):
                        pt = psT.tile([128, 512], bf16, tag="trps")
                        for w in range(4):
                            ni = 4 * ch + w
                            nc.tensor.transpose(
                                pt[:, 128 * w:128 * (w + 1)],
                                o[:, ni, 2 * tpi:2 * tpi + 2, :]
                                .rearrange("p t d -> p (t d)"),
                                ident[:])
                        ofm = wp.tile([128, 512], bf16, tag="ofm")
                        nc.scalar.copy(ofm[:], pt[:])
                        po = psA.tile([128, 512], f32, tag="mmps", name="po")
                        nc.tensor.matmul(po[:], Wo_bd[l][:], ofm[:],
                                         start=True, stop=True)
                        g2s = g2[:, tpi, 512 * ch:512 * (ch + 1)]
                        # x1 = G2 + (wo_out + bo)   (in-place)
                        nc.vector.scalar_tensor_tensor(
                            g2s, po[:], bov[l][:, 0:1], g2s, op0=AL.add, op1=AL.add)
                        x1ns = X1N[:, tpi, 512 * ch:512 * (ch + 1)]
                        _ln_fm(g2s, x1ns, g1v[l], be1v[l], "1")
                        # FFN
                        pz = psZ.tile([128, 512], f32, tag="zps")
                        for c in range(4):
                            pmid = psA.tile([128, 512], f32, tag="mmps", name="pmid")
                            nc.tensor.matmul(pmid[:], W1c[l][:, c], x1ns,
                                             start=True, stop=True)
                            mid = p3.tile([128, 512], bf16, tag="mid")
                            nc.scalar.activation(mid[:], pmid[:], AF.Relu,
                                                 bias=b1v[l][:, c % 2:c % 2 + 1])
                            nc.tensor.matmul(pz[:], W2c[l][:, c], mid[:],
                                             start=(c == 0), stop=(c == 3))
                        # z = x1n + (w2_out + b2)  (stored into G2 slot)
                        nc.vector.scalar_tensor_tensor(
                            g2s, pz[:], b2v[l][:, 0:1], x1ns, op0=AL.add, op1=AL.add)
                        # LN2 -> X (fm)
                        _ln_fm(g2s, X[:, bi, tpi, 512 * ch:512 * (ch + 1)],
                               g2v[l], be2v[l], "2")
                tx = tap(f"X{l}b{bi}", [128, TP * N])
                if tx is not None:
                    nc.sync.dma_start(
                        tx.ap(), X[:, bi].rearrange("p tp n -> p (tp n)"))

            def stage_F(bi):
                outsb = wp.tile([OS, N], bf16, tag="outsb")
                for ch in range(2):
                    pf = psA.tile([128, 512], f32, tag="mmps", name="pf_out")[:OS]
                    for tpi in range(TP):
                        nc.tensor.matmul(
                            pf, Woutc[:, tpi],
                            X[:, bi, tpi, 512 * ch:512 * (ch + 1)],
                            start=(tpi == 0), stop=(tpi == TP - 1))
                    nc.scalar.activation(outsb[:, 512 * ch:512 * (ch + 1)], pf,
                                         AF.Identity, bias=boutv[:, 0:1])
                nc.gpsimd.dma_start(out_d.ap()[bi].rearrange("s n o -> s (n o)"),
                                    outsb[:])

            # -------- emission: 2-stream interleave --------
            prog = {b: [] for b in range(BL)}
            for l in range(L):
                for b in range(BL):
                    prog[b] += [(stage_A, l, b), (stage_QC, l, b), (stage_D, l, b)]
            for b in range(BL):
                prog[b].append((stage_F, b))
            if CFG["seq"]:
                order = prog[0] + prog[1]
            else:
                order = []
                i0 = i1 = 0
                OFFSET = CFG["offset"]
                while i0 < len(prog[0]) or i1 < len(prog[1]):
                    if i0 < len(prog[0]) and (i0 - OFFSET < i1 or i1 >= len(prog[1])):
                        order.append(prog[0][i0]); i0 += 1
                    else:
                        order.append(prog[1][i1]); i1 += 1
            for fn, *args in order:
                fn(*args)

    nc.compile()
    return nc, taps


_CACHE = {}


def _get_nc(qkv_bias=False):
    key = ("nc", qkv_bias)
    if key not in _CACHE:
        _CACHE[key] = build_nc(qkv_bias)
    return _CACHE[key]


def _prep_inputs(inputs):
    import ml_dtypes
    bf = ml_dtypes.bfloat16
    x = np.asarray(inputs["x"], dtype=np.float32)         # [B, T, N, D]
    # fm layout: [B, (j=t%2, d), tp, n]
    x_fm = np.ascontiguousarray(
        x.reshape(B, TP, 2, N, D).transpose(0, 2, 4, 1, 3)
        .reshape(B, 128, TP, N)).astype(bf)
    sup = np.asarray(inputs["supports"], dtype=np.float32)
    a_bf = np.ascontiguousarray(sup).astype(bf)
    at_bf = np.ascontiguousarray(sup.transpose(0, 2, 1)).astype(bf)
    bqkv = np.stack([np.asarray(inputs["bq"], np.float32),
                     np.asarray(inputs["bk"], np.float32),
                     np.asarray(inputs["bv"], np.float32)], axis=1)  # [L,3,D]
    shared = {"a_bf": a_bf, "at_bf": at_bf,
              "bqkv": np.ascontiguousarray(bqkv)}
    names = ["Wg", "bg", "Wq", "Wk", "Wv", "Wo", "bo", "W1", "b1", "W2", "b2",
             "ln1_g", "ln1_b", "ln2_g", "ln2_b", "Wout", "bout"]
    for n in names:
        shared[n] = np.ascontiguousarray(np.asarray(inputs[n], dtype=np.float32))
    qkv_bias = bool(np.any(bqkv))
    in_maps = []
    for c in range(NCORES):
        m = dict(shared)
        m["x"] = np.ascontiguousarray(x_fm[c * BL:(c + 1) * BL])
        in_maps.append(m)
    return in_maps, qkv_bias


def kernel(**inputs):
    from concourse.bass_utils import run_bass_kernel_spmd
    in_maps, qkv_bias = _prep_inputs(inputs)
    nc, taps = _get_nc(qkv_bias)
    res = run_bass_kernel_spmd(nc, in_maps, core_ids=list(range(NCORES)))
    _CACHE["last_res"] = res
    out = np.concatenate([r["out"] for r in res.results], axis=0)
    return out.astype(np.float32)
